# revision 4
# baseline (speedup 1.0000x reference)
"""Dense graph-attention layer (GAT) on 8 Trainium2 NeuronCores.

Reference computation (all f32):
    h = x @ W                      # [N, F_OUT]
    f_src = h @ a_src              # [N]
    f_dst = h @ a_dst              # [N]
    e[i,j] = leaky_relu(f_src[i] + f_dst[j], 0.2), masked to -inf where adj==0
    alpha = softmax(e, axis=1)
    out = alpha @ h                # [N, F_OUT]

Sharding: rows i are sharded across 8 cores (1024 rows each). Each core
receives its slice of adj pre-transposed to [N, 1024] (j on partitions once
tiled) and encoded as adjm = (adj - 1) * 150, i.e. 0 for edges and -150 for
non-edges, so masking is a plain add folded into the attention logits:
exp(e - 150) ~ 0. leaky_relu is folded into the exp via
exp(leaky(e)) = max(exp(e), exp(0.2 e)) (exp is monotone).

Per j-tile [128, 1024] the device computes in PSUM
    t = adjm + f_src_row (rank-1 PE matmul on top of an identity-matmul copy)
then u = Exp(t + f_dst[j]) and v = Exp(0.2 t + 0.2 f_dst[j]) on ScalarE
(per-partition bias), s = max(u, v) on VectorE, and accumulates
    outT += h_tile.T @ s   (PE, contraction over j)
    denom += s             (VectorE, row sums finished by a ones-matmul)
Finally outT is transposed back and scaled by 1/denom.
"""

import numpy as np
from contextlib import ExitStack

import concourse.bacc as bacc
import concourse.tile as tile
from concourse import mybir
from concourse.bass_utils import run_bass_kernel_spmd

F32 = mybir.dt.float32
AF = mybir.ActivationFunctionType
OP = mybir.AluOpType

N = 8192
F_IN = 256
F_OUT = 128
N_CORES = 8
ROWS = N // N_CORES          # 1024 rows of the output per core
P = 128                      # partitions
JT = N // P                  # 64 j-tiles per core
IT = ROWS // P               # 8 i-tiles per core
SLOPE = 0.2
MASK = 150.0                 # additive mask magnitude: exp(-150)==0 in f32

LAST_EXEC_TIME_NS = None


def _build_program():
    nc = bacc.Bacc("TRN2", target_bir_lowering=False, debug=False,
                   num_devices=N_CORES)

    adjm = nc.dram_tensor("adjm", [N, ROWS], F32, kind="ExternalInput")
    xT = nc.dram_tensor("xT", [F_IN, N], F32, kind="ExternalInput")
    xoT = nc.dram_tensor("xoT", [F_IN, ROWS], F32, kind="ExternalInput")
    w_in = nc.dram_tensor("W", [F_IN, F_OUT], F32, kind="ExternalInput")
    asrc = nc.dram_tensor("a_src", [F_OUT, 1], F32, kind="ExternalInput")
    adst = nc.dram_tensor("a_dst", [F_OUT, 1], F32, kind="ExternalInput")
    ident = nc.dram_tensor("ident", [P, P], F32, kind="ExternalInput")
    ones_r = nc.dram_tensor("ones_r", [1, P], F32, kind="ExternalInput")
    ones_c = nc.dram_tensor("ones_c", [P, 1], F32, kind="ExternalInput")
    out = nc.dram_tensor("out", [ROWS, F_OUT], F32, kind="ExternalOutput")
    # ExternalOutput used as DRAM scratch: internal DRAM allocations fail
    # to load under the axon PJRT path.
    # 2-D shapes: 1-D DRAM outputs fail to load under the axon PJRT path.
    fd_dram = nc.dram_tensor("fd_scratch", [1, N], F32, kind="ExternalOutput")
    den_dram = nc.dram_tensor("den_scratch", [1, ROWS], F32, kind="ExternalOutput")

    with tile.TileContext(nc) as tc:
        with ExitStack() as ctx:
            persist = ctx.enter_context(tc.tile_pool(name="persist", bufs=1))
            opsum = ctx.enter_context(
                tc.tile_pool(name="opsum", bufs=1, space="PSUM"))

            h_sb = persist.tile([P, JT * P], F32)      # h[j,f] per j-block
            fs_row = persist.tile([1, ROWS], F32)      # f_src (own rows)
            fd_col = persist.tile([P, JT], F32)        # f_dst, per-partition
            fd2_col = persist.tile([P, JT], F32)       # 0.2 * f_dst
            acc = persist.tile([P, ROWS], F32)         # running sum of s
            id_sb = persist.tile([P, P], F32)
            ones_r_sb = persist.tile([1, P], F32)
            ones_c_sb = persist.tile([P, 1], F32)
            w_sb = persist.tile([P, 2 * F_OUT], F32)   # W k-halves side by side
            asrc_sb = persist.tile([F_OUT, 1], F32)
            adst_sb = persist.tile([F_OUT, 1], F32)
            inv_col = persist.tile([P, IT], F32)

            nc.sync.dma_start(id_sb[:], ident[:, :])
            nc.sync.dma_start(ones_r_sb[:], ones_r[:, :])
            nc.sync.dma_start(ones_c_sb[:], ones_c[:, :])
            nc.sync.dma_start(w_sb[:, 0:F_OUT], w_in[0:P, :])
            nc.sync.dma_start(w_sb[:, F_OUT:2 * F_OUT], w_in[P:2 * P, :])
            nc.sync.dma_start(asrc_sb[:], asrc[:, :])
            nc.sync.dma_start(adst_sb[:], adst[:, :])

            # ---------------- prep: h, hT, f_src, f_dst ----------------
            with ExitStack() as pctx:
                prep = pctx.enter_context(tc.tile_pool(name="prep", bufs=1))
                ppsum = pctx.enter_context(
                    tc.tile_pool(name="ppsum", bufs=2, space="PSUM"))

                xt_sb = prep.tile([P, 2 * N], F32)     # xT k-halves
                nc.sync.dma_start(xt_sb[:, 0:N], xT[0:P, :])
                nc.sync.dma_start(xt_sb[:, N:2 * N], xT[P:2 * P, :])
                xo_sb = prep.tile([P, 2 * ROWS], F32)  # own-slice xT k-halves
                nc.sync.dma_start(xo_sb[:, 0:ROWS], xoT[0:P, :])
                nc.sync.dma_start(xo_sb[:, ROWS:2 * ROWS], xoT[P:2 * P, :])

                # h[j,f] blocks (lhsT for the output matmul): 4 blocks/psum
                for q in range(JT // 4):
                    ph = ppsum.tile([P, 512], F32, tag="pp")
                    for u4 in range(4):
                        jt = q * 4 + u4
                        psl = ph[:, u4 * P:(u4 + 1) * P]
                        nc.tensor.matmul(
                            psl, lhsT=xt_sb[:, jt * P:(jt + 1) * P],
                            rhs=w_sb[:, 0:F_OUT], start=True, stop=False)
                        nc.tensor.matmul(
                            psl, lhsT=xt_sb[:, N + jt * P:N + (jt + 1) * P],
                            rhs=w_sb[:, F_OUT:2 * F_OUT], start=False, stop=True)
                    dst = h_sb[:, q * 512:(q + 1) * 512]
                    if q % 2 == 0:
                        nc.scalar.copy(dst, ph[:])
                    else:
                        nc.vector.tensor_copy(dst, ph[:])

                # hT[f,j] (for f_dst)
                hT_sb = prep.tile([P, N], F32)
                for q in range(N // 512):
                    pt = ppsum.tile([P, 512], F32, tag="pp")
                    nc.tensor.matmul(
                        pt[:], lhsT=w_sb[:, 0:F_OUT],
                        rhs=xt_sb[:, q * 512:(q + 1) * 512],
                        start=True, stop=False)
                    nc.tensor.matmul(
                        pt[:], lhsT=w_sb[:, F_OUT:2 * F_OUT],
                        rhs=xt_sb[:, N + q * 512:N + (q + 1) * 512],
                        start=False, stop=True)
                    dst = hT_sb[:, q * 512:(q + 1) * 512]
                    if q % 2 == 0:
                        nc.scalar.copy(dst, pt[:])
                    else:
                        nc.vector.tensor_copy(dst, pt[:])

                # f_dst row [1, N] -> DRAM -> per-partition layout [128, 64]
                fd_sbuf = prep.tile([1, N], F32)
                for q in range(N // 512):
                    pf = ppsum.tile([1, 512], F32, tag="pp")
                    nc.tensor.matmul(
                        pf[:], lhsT=adst_sb[:],
                        rhs=hT_sb[:, q * 512:(q + 1) * 512],
                        start=True, stop=True)
                    nc.scalar.copy(fd_sbuf[:, q * 512:(q + 1) * 512], pf[:])
                nc.sync.dma_start(fd_dram[:, :], fd_sbuf[:])
                nc.sync.dma_start(
                    fd_col[:], fd_dram.ap()[0].rearrange("(jt p) -> p jt", p=P))
                nc.vector.tensor_scalar_mul(fd2_col[:], fd_col[:], SLOPE)

                # h_ownT[f,i] then f_src row [1, ROWS] (stays in SBUF)
                hown_sb = prep.tile([P, ROWS], F32)
                for q in range(ROWS // 512):
                    po = ppsum.tile([P, 512], F32, tag="pp")
                    nc.tensor.matmul(
                        po[:], lhsT=w_sb[:, 0:F_OUT],
                        rhs=xo_sb[:, q * 512:(q + 1) * 512],
                        start=True, stop=False)
                    nc.tensor.matmul(
                        po[:], lhsT=w_sb[:, F_OUT:2 * F_OUT],
                        rhs=xo_sb[:, ROWS + q * 512:ROWS + (q + 1) * 512],
                        start=False, stop=True)
                    nc.scalar.copy(hown_sb[:, q * 512:(q + 1) * 512], po[:])
                for q in range(ROWS // 512):
                    pfs = ppsum.tile([1, 512], F32, tag="pp")
                    nc.tensor.matmul(
                        pfs[:], lhsT=asrc_sb[:],
                        rhs=hown_sb[:, q * 512:(q + 1) * 512],
                        start=True, stop=True)
                    nc.scalar.copy(fs_row[:, q * 512:(q + 1) * 512], pfs[:])

            # ---------------- main loop over 64 j-tiles ----------------
            with ExitStack() as mctx:
                adj_pool = mctx.enter_context(tc.tile_pool(name="adj", bufs=3))
                uv_pool = mctx.enter_context(tc.tile_pool(name="uv", bufs=4))
                s_pool = mctx.enter_context(tc.tile_pool(name="s", bufs=3))
                tpsum = mctx.enter_context(
                    tc.tile_pool(name="tpsum", bufs=2, space="PSUM"))

                psum_out = opsum.tile([P, ROWS], F32)   # outT accumulator

                for jt in range(JT):
                    at = adj_pool.tile([P, ROWS], F32, tag="adj")
                    nc.sync.dma_start(at[:], adjm[jt * P:(jt + 1) * P, :])

                    pt = tpsum.tile([P, ROWS], F32, tag="t")
                    for hh in range(2):
                        sl = slice(hh * 512, (hh + 1) * 512)
                        nc.tensor.matmul(pt[:, sl], lhsT=id_sb[:],
                                         rhs=at[:, sl], start=True, stop=False)
                        nc.tensor.matmul(pt[:, sl], lhsT=ones_r_sb[:],
                                         rhs=fs_row[:, sl], start=False,
                                         stop=True)

                    u = uv_pool.tile([P, ROWS], F32, tag="u")
                    nc.scalar.activation(u[:], pt[:], AF.Exp,
                                         bias=fd_col[:, jt:jt + 1], scale=1.0)
                    v = uv_pool.tile([P, ROWS], F32, tag="v")
                    nc.scalar.activation(v[:], pt[:], AF.Exp,
                                         bias=fd2_col[:, jt:jt + 1], scale=SLOPE)

                    s = s_pool.tile([P, ROWS], F32, tag="s")
                    nc.vector.tensor_tensor(s[:], u[:], v[:], op=OP.max)

                    for hh in range(2):
                        sl = slice(hh * 512, (hh + 1) * 512)
                        nc.tensor.matmul(
                            psum_out[:, sl],
                            lhsT=h_sb[:, jt * P:(jt + 1) * P],
                            rhs=s[:, sl],
                            start=(jt == 0), stop=(jt == JT - 1))

                    if jt == 0:
                        nc.vector.tensor_copy(acc[:], s[:])
                    else:
                        nc.vector.tensor_add(acc[:], acc[:], s[:])

            # ---------------- epilogue: denom, normalize, transpose ----
            with ExitStack() as ectx:
                epi = ectx.enter_context(tc.tile_pool(name="epi", bufs=2))
                epsum = ectx.enter_context(
                    tc.tile_pool(name="epsum", bufs=2, space="PSUM"))

                den_row = epi.tile([1, ROWS], F32, tag="den")
                for q in range(ROWS // 512):
                    pd = epsum.tile([1, 512], F32, tag="ep")
                    nc.tensor.matmul(pd[:], lhsT=ones_c_sb[:],
                                     rhs=acc[:, q * 512:(q + 1) * 512],
                                     start=True, stop=True)
                    nc.scalar.copy(den_row[:, q * 512:(q + 1) * 512], pd[:])
                nc.sync.dma_start(den_dram[:, :], den_row[:])
                den_col = epi.tile([P, IT], F32, tag="denc")
                nc.sync.dma_start(
                    den_col[:], den_dram.ap()[0].rearrange("(it p) -> p it", p=P))
                nc.vector.reciprocal(inv_col[:], den_col[:])

                outT_sb = epi.tile([P, ROWS], F32, tag="outT")
                nc.scalar.copy(outT_sb[:], psum_out[:])
                for it in range(IT):
                    ptr = epsum.tile([P, P], F32, tag="ep")
                    nc.tensor.transpose(
                        ptr[:], outT_sb[:, it * P:(it + 1) * P], id_sb[:])
                    ot = epi.tile([P, P], F32, tag="ot")
                    nc.vector.tensor_scalar_mul(
                        ot[:], ptr[:], inv_col[:, it:it + 1])
                    nc.sync.dma_start(out[it * P:(it + 1) * P, :], ot[:])

    nc.compile()
    return nc


_PROGRAM = None


def _get_program():
    global _PROGRAM
    if _PROGRAM is None:
        _PROGRAM = _build_program()
    return _PROGRAM


def kernel(x, adj, W, a_src, a_dst):
    global LAST_EXEC_TIME_NS
    x = np.asarray(x, dtype=np.float32)
    adj = np.asarray(adj, dtype=np.float32)
    W = np.asarray(W, dtype=np.float32)
    a_src = np.asarray(a_src, dtype=np.float32).reshape(F_OUT, 1)
    a_dst = np.asarray(a_dst, dtype=np.float32).reshape(F_OUT, 1)

    nc = _get_program()

    xT = np.ascontiguousarray(x.T)
    ident = np.eye(P, dtype=np.float32)
    ones_r = np.ones((1, P), dtype=np.float32)
    ones_c = np.ones((P, 1), dtype=np.float32)

    in_maps = []
    for c in range(N_CORES):
        rows = slice(c * ROWS, (c + 1) * ROWS)
        adjm = np.ascontiguousarray((adj[rows, :].T - 1.0) * MASK)
        xoT = np.ascontiguousarray(x[rows, :].T)
        in_maps.append({
            "adjm": adjm,
            "xT": xT,
            "xoT": xoT,
            "W": W,
            "a_src": a_src,
            "a_dst": a_dst,
            "ident": ident,
            "ones_r": ones_r,
            "ones_c": ones_c,
        })

    res = run_bass_kernel_spmd(nc, in_maps, core_ids=list(range(N_CORES)))
    LAST_EXEC_TIME_NS = res.exec_time_ns
    return np.concatenate(
        [res.results[c]["out"] for c in range(N_CORES)], axis=0)


# revision 7
# speedup vs baseline: 3.1844x; 3.1844x over previous
"""Dense graph-attention layer (GAT) on 8 Trainium2 NeuronCores.

Reference computation (all f32):
    h = x @ W                      # [N, F_OUT]
    f_src = h @ a_src              # [N]
    f_dst = h @ a_dst              # [N]
    e[i,j] = leaky_relu(f_src[i] + f_dst[j], 0.2), masked to -inf where adj==0
    alpha = softmax(e, axis=1)
    out = alpha @ h                # [N, F_OUT]

Sharding: output rows i are sharded across 8 cores (1024 rows each). Each
core receives its slice of adj transposed to [N, 1024] as a bf16 0/1 mask,
so j lands on partitions when tiled — the orientation the alpha @ h
contraction needs.

Key reformulation (exact, not approximate): with softmax's invariance to a
per-row scale, exp(f_src[i]) factors out of both the numerator and the
denominator and cancels. Writing B = exp(f_dst), D = exp(0.2 f_dst),
c = exp(-0.8 f_src):
    exp(leaky_relu(e)) = max(exp(e), exp(0.2 e))          (exp is monotone)
                       = exp(f_src) * max(B[j], c[i] * D[j])
so alpha rows can be computed from s[j,i] = mask[j,i] * max(B[j], c[i]*D[j])
directly. No per-element exp/activation is needed at all — only a dual-op
tensor_scalar (mult+max against two per-partition vectors) and a mask
multiply, both on VectorE in bf16. PE accumulates outT += h_tile.T @ s and
denom += ones.T @ s across all 64 j-tiles in PSUM. exp() runs only on the
tiny f_src/f_dst vectors.
"""

import numpy as np
import ml_dtypes
from contextlib import ExitStack

import concourse.bacc as bacc
import concourse.tile as tile
from concourse import mybir
from concourse.bass_utils import run_bass_kernel_spmd

F32 = mybir.dt.float32
BF16 = mybir.dt.bfloat16
AF = mybir.ActivationFunctionType
OP = mybir.AluOpType

N = 8192
F_IN = 256
F_OUT = 128
N_CORES = 8
ROWS = N // N_CORES          # 1024 output rows per core
P = 128                      # partitions
JT = N // P                  # 64 j-tiles per core
IT = ROWS // P               # 8 i-tiles per core
SLOPE = 0.2

# Fraction of mask-multiplies routed to GpSimd instead of VectorE (tunable).
GPSIMD_EVERY = 0             # 0 = disabled; k>0 = every k-th tile

LAST_EXEC_TIME_NS = None
LAST_RESULT = None


def _build_program():
    nc = bacc.Bacc("TRN2", target_bir_lowering=False, debug=False,
                   num_devices=N_CORES)

    mask = nc.dram_tensor("mask", [N, ROWS], BF16, kind="ExternalInput")
    xT = nc.dram_tensor("xT", [F_IN, N], BF16, kind="ExternalInput")
    xoT = nc.dram_tensor("xoT", [F_IN, ROWS], BF16, kind="ExternalInput")
    w_in = nc.dram_tensor("W", [F_IN, F_OUT], BF16, kind="ExternalInput")
    asrc = nc.dram_tensor("a_src", [F_OUT, 1], BF16, kind="ExternalInput")
    adst = nc.dram_tensor("a_dst", [F_OUT, 1], BF16, kind="ExternalInput")
    ident = nc.dram_tensor("ident", [P, P], F32, kind="ExternalInput")
    ones_rb = nc.dram_tensor("ones_rb", [1, P], BF16, kind="ExternalInput")
    ones_cb = nc.dram_tensor("ones_cb", [P, 1], BF16, kind="ExternalInput")
    out = nc.dram_tensor("out", [ROWS, F_OUT], F32, kind="ExternalOutput")
    # 2-D shapes: 1-D DRAM outputs fail to load under the axon PJRT path.
    # ExternalOutput doubles as DRAM scratch (Internal DRAM also fails).
    fd_dram = nc.dram_tensor("fd_scratch", [1, N], F32, kind="ExternalOutput")
    den_dram = nc.dram_tensor("den_scratch", [1, ROWS], F32,
                              kind="ExternalOutput")

    with tile.TileContext(nc) as tc:
        with ExitStack() as ctx:
            persist = ctx.enter_context(tc.tile_pool(name="persist", bufs=1))
            opsum = ctx.enter_context(
                tc.tile_pool(name="opsum", bufs=1, space="PSUM"))

            h_sb = persist.tile([P, JT * P], BF16)     # h[j,f] per j-block
            c_bcast = persist.tile([P, ROWS], BF16)    # exp(-0.8 f_src) bcast
            b_col = persist.tile([P, JT], F32)         # exp(f_dst)
            d_col = persist.tile([P, JT], F32)         # exp(0.2 f_dst)
            id_sb = persist.tile([P, P], F32)
            ones_r_sb = persist.tile([1, P], BF16)
            ones_c_sb = persist.tile([P, 1], BF16)
            w_sb = persist.tile([P, 2 * F_OUT], BF16)  # W k-halves
            asrc_sb = persist.tile([F_OUT, 1], BF16)
            adst_sb = persist.tile([F_OUT, 1], BF16)
            inv_col = persist.tile([P, IT], F32)

            nc.sync.dma_start(id_sb[:], ident[:, :])
            nc.sync.dma_start(ones_r_sb[:], ones_rb[:, :])
            nc.sync.dma_start(ones_c_sb[:], ones_cb[:, :])
            nc.sync.dma_start(w_sb[:, 0:F_OUT], w_in[0:P, :])
            nc.sync.dma_start(w_sb[:, F_OUT:2 * F_OUT], w_in[P:2 * P, :])
            nc.sync.dma_start(asrc_sb[:], asrc[:, :])
            nc.sync.dma_start(adst_sb[:], adst[:, :])

            # ---------------- prep: h, hT, f_src, f_dst ----------------
            with ExitStack() as pctx:
                prep = pctx.enter_context(tc.tile_pool(name="prep", bufs=1))
                ppsum = pctx.enter_context(
                    tc.tile_pool(name="ppsum", bufs=2, space="PSUM"))

                xt_sb = prep.tile([P, 2 * N], BF16)    # xT k-halves
                nc.sync.dma_start(xt_sb[:, 0:N], xT[0:P, :])
                nc.sync.dma_start(xt_sb[:, N:2 * N], xT[P:2 * P, :])
                xo_sb = prep.tile([P, 2 * ROWS], BF16)
                nc.sync.dma_start(xo_sb[:, 0:ROWS], xoT[0:P, :])
                nc.sync.dma_start(xo_sb[:, ROWS:2 * ROWS], xoT[P:2 * P, :])

                # h[j,f] blocks (lhsT for the output matmul): 4 blocks/psum
                for q in range(JT // 4):
                    ph = ppsum.tile([P, 512], F32, tag="pp")
                    for u4 in range(4):
                        jt = q * 4 + u4
                        psl = ph[:, u4 * P:(u4 + 1) * P]
                        nc.tensor.matmul(
                            psl, lhsT=xt_sb[:, jt * P:(jt + 1) * P],
                            rhs=w_sb[:, 0:F_OUT], start=True, stop=False)
                        nc.tensor.matmul(
                            psl, lhsT=xt_sb[:, N + jt * P:N + (jt + 1) * P],
                            rhs=w_sb[:, F_OUT:2 * F_OUT], start=False,
                            stop=True)
                    dst = h_sb[:, q * 512:(q + 1) * 512]
                    if q % 2 == 0:
                        nc.scalar.copy(dst, ph[:])
                    else:
                        nc.vector.tensor_copy(dst, ph[:])

                # hT[f,j] (only to compute f_dst)
                hT_sb = prep.tile([P, N], BF16)
                for q in range(N // 512):
                    pt = ppsum.tile([P, 512], F32, tag="pp")
                    nc.tensor.matmul(
                        pt[:], lhsT=w_sb[:, 0:F_OUT],
                        rhs=xt_sb[:, q * 512:(q + 1) * 512],
                        start=True, stop=False)
                    nc.tensor.matmul(
                        pt[:], lhsT=w_sb[:, F_OUT:2 * F_OUT],
                        rhs=xt_sb[:, N + q * 512:N + (q + 1) * 512],
                        start=False, stop=True)
                    dst = hT_sb[:, q * 512:(q + 1) * 512]
                    if q % 2 == 0:
                        nc.scalar.copy(dst, pt[:])
                    else:
                        nc.vector.tensor_copy(dst, pt[:])

                # f_dst row [1, N] -> DRAM -> per-partition layout [128, 64]
                fd_sbuf = prep.tile([1, N], F32)
                for q in range(N // 512):
                    pf = ppsum.tile([1, 512], F32, tag="pp")
                    nc.tensor.matmul(
                        pf[:], lhsT=adst_sb[:],
                        rhs=hT_sb[:, q * 512:(q + 1) * 512],
                        start=True, stop=True)
                    nc.scalar.copy(fd_sbuf[:, q * 512:(q + 1) * 512], pf[:])
                nc.sync.dma_start(fd_dram[:, :], fd_sbuf[:])
                fd_col = prep.tile([P, JT], F32)
                nc.sync.dma_start(
                    fd_col[:], fd_dram.ap()[0].rearrange("(jt p) -> p jt", p=P))
                nc.scalar.activation(b_col[:], fd_col[:], AF.Exp)
                nc.scalar.activation(d_col[:], fd_col[:], AF.Exp, scale=SLOPE)

                # h_ownT[f,i] -> f_src row -> c = exp(-0.8 f_src), broadcast
                hown_sb = prep.tile([P, ROWS], BF16)
                for q in range(ROWS // 512):
                    po = ppsum.tile([P, 512], F32, tag="pp")
                    nc.tensor.matmul(
                        po[:], lhsT=w_sb[:, 0:F_OUT],
                        rhs=xo_sb[:, q * 512:(q + 1) * 512],
                        start=True, stop=False)
                    nc.tensor.matmul(
                        po[:], lhsT=w_sb[:, F_OUT:2 * F_OUT],
                        rhs=xo_sb[:, ROWS + q * 512:ROWS + (q + 1) * 512],
                        start=False, stop=True)
                    nc.scalar.copy(hown_sb[:, q * 512:(q + 1) * 512], po[:])
                c_row = prep.tile([1, ROWS], BF16)
                for q in range(ROWS // 512):
                    pfs = ppsum.tile([1, 512], F32, tag="pp")
                    nc.tensor.matmul(
                        pfs[:], lhsT=asrc_sb[:],
                        rhs=hown_sb[:, q * 512:(q + 1) * 512],
                        start=True, stop=True)
                    nc.scalar.activation(c_row[:, q * 512:(q + 1) * 512],
                                         pfs[:], AF.Exp, scale=-0.8)
                for q in range(ROWS // 512):
                    pcb = ppsum.tile([P, 512], F32, tag="pp")
                    nc.tensor.matmul(
                        pcb[:], lhsT=ones_r_sb[:],
                        rhs=c_row[:, q * 512:(q + 1) * 512],
                        start=True, stop=True)
                    nc.scalar.copy(c_bcast[:, q * 512:(q + 1) * 512], pcb[:])

            # ---------------- main loop over 64 j-tiles ----------------
            with ExitStack() as mctx:
                msk_pool = mctx.enter_context(tc.tile_pool(name="msk", bufs=4))
                m_pool = mctx.enter_context(tc.tile_pool(name="m", bufs=3))
                s_pool = mctx.enter_context(tc.tile_pool(name="s", bufs=3))

                psum_out = opsum.tile([P, ROWS], F32)   # outT accumulator
                psum_den = opsum.tile([1, ROWS], F32)   # denom accumulator

                for jt in range(JT):
                    mk = msk_pool.tile([P, ROWS], BF16, tag="mk")
                    nc.sync.dma_start(mk[:], mask[jt * P:(jt + 1) * P, :])

                    m = m_pool.tile([P, ROWS], BF16, tag="m")
                    nc.vector.tensor_scalar(
                        m[:], c_bcast[:], d_col[:, jt:jt + 1],
                        b_col[:, jt:jt + 1], op0=OP.mult, op1=OP.max)

                    s = s_pool.tile([P, ROWS], BF16, tag="s")
                    if GPSIMD_EVERY and jt % GPSIMD_EVERY == 0:
                        nc.gpsimd.tensor_tensor(s[:], m[:], mk[:], op=OP.mult)
                    else:
                        nc.vector.tensor_tensor(s[:], m[:], mk[:], op=OP.mult)

                    for hh in range(2):
                        sl = slice(hh * 512, (hh + 1) * 512)
                        nc.tensor.matmul(
                            psum_out[:, sl],
                            lhsT=h_sb[:, jt * P:(jt + 1) * P],
                            rhs=s[:, sl],
                            start=(jt == 0), stop=(jt == JT - 1))
                        nc.tensor.matmul(
                            psum_den[:, sl], lhsT=ones_c_sb[:], rhs=s[:, sl],
                            start=(jt == 0), stop=(jt == JT - 1))

            # ---------------- epilogue: normalize + transpose ----------
            with ExitStack() as ectx:
                epi = ectx.enter_context(tc.tile_pool(name="epi", bufs=2))
                epsum = ectx.enter_context(
                    tc.tile_pool(name="epsum", bufs=2, space="PSUM"))

                den_row = epi.tile([1, ROWS], F32, tag="den")
                nc.scalar.copy(den_row[:], psum_den[:])
                nc.sync.dma_start(den_dram[:, :], den_row[:])
                den_col = epi.tile([P, IT], F32, tag="denc")
                nc.sync.dma_start(
                    den_col[:], den_dram.ap()[0].rearrange("(it p) -> p it", p=P))
                nc.vector.reciprocal(inv_col[:], den_col[:])

                outT_sb = epi.tile([P, ROWS], F32, tag="outT")
                nc.scalar.copy(outT_sb[:], psum_out[:])
                for it in range(IT):
                    ptr = epsum.tile([P, P], F32, tag="ep")
                    nc.tensor.transpose(
                        ptr[:], outT_sb[:, it * P:(it + 1) * P], id_sb[:])
                    ot = epi.tile([P, P], F32, tag="ot")
                    nc.vector.tensor_scalar_mul(
                        ot[:], ptr[:], inv_col[:, it:it + 1])
                    nc.sync.dma_start(out[it * P:(it + 1) * P, :], ot[:])

    nc.compile()
    return nc


_PROGRAM = None


def _get_program():
    global _PROGRAM
    if _PROGRAM is None:
        _PROGRAM = _build_program()
    return _PROGRAM


def kernel(x, adj, W, a_src, a_dst):
    global LAST_EXEC_TIME_NS, LAST_RESULT
    x = np.asarray(x, dtype=np.float32)
    adj = np.asarray(adj, dtype=np.float32)
    W = np.asarray(W, dtype=np.float32)
    a_src = np.asarray(a_src, dtype=np.float32).reshape(F_OUT, 1)
    a_dst = np.asarray(a_dst, dtype=np.float32).reshape(F_OUT, 1)

    nc = _get_program()

    bf = ml_dtypes.bfloat16
    xT = np.ascontiguousarray(x.T).astype(bf)
    in_common = {
        "xT": xT,
        "W": W.astype(bf),
        "a_src": a_src.astype(bf),
        "a_dst": a_dst.astype(bf),
        "ident": np.eye(P, dtype=np.float32),
        "ones_rb": np.ones((1, P), dtype=bf),
        "ones_cb": np.ones((P, 1), dtype=bf),
    }
    in_maps = []
    for c in range(N_CORES):
        rows = slice(c * ROWS, (c + 1) * ROWS)
        im = dict(in_common)
        im["mask"] = np.ascontiguousarray(adj[rows, :].T).astype(bf)
        im["xoT"] = np.ascontiguousarray(x[rows, :].T).astype(bf)
        in_maps.append(im)

    res = run_bass_kernel_spmd(nc, in_maps, core_ids=list(range(N_CORES)))
    LAST_EXEC_TIME_NS = res.exec_time_ns
    LAST_RESULT = res
    return np.concatenate(
        [res.results[c]["out"] for c in range(N_CORES)], axis=0)


# revision 9
# speedup vs baseline: 3.2250x; 1.0127x over previous
"""Dense graph-attention layer (GAT) on 8 Trainium2 NeuronCores.

Reference computation (all f32):
    h = x @ W                      # [N, F_OUT]
    f_src = h @ a_src              # [N]
    f_dst = h @ a_dst              # [N]
    e[i,j] = leaky_relu(f_src[i] + f_dst[j], 0.2), masked to -inf where adj==0
    alpha = softmax(e, axis=1)
    out = alpha @ h                # [N, F_OUT]

Sharding: output rows i are sharded across 8 cores (1024 rows each). Each
core receives its slice of adj transposed to [N, 1024] as a bf16 0/1 mask,
so j lands on partitions when tiled — the orientation the alpha @ h
contraction needs.

Key reformulation (exact, not approximate): with softmax's invariance to a
per-row scale, exp(f_src[i]) factors out of both the numerator and the
denominator and cancels. Writing B = exp(f_dst), D = exp(0.2 f_dst),
c = exp(-0.8 f_src):
    exp(leaky_relu(e)) = max(exp(e), exp(0.2 e))          (exp is monotone)
                       = exp(f_src) * max(B[j], c[i] * D[j])
so alpha rows can be computed from s[j,i] = mask[j,i] * max(B[j], c[i]*D[j])
directly. No per-element exp/activation is needed at all — only a dual-op
tensor_scalar (mult+max against two per-partition vectors) and a mask
multiply, both on VectorE in bf16. PE accumulates outT += h_tile.T @ s and
denom += ones.T @ s across all 64 j-tiles in PSUM. exp() runs only on the
tiny f_src/f_dst vectors.
"""

import numpy as np
import ml_dtypes
from contextlib import ExitStack

import concourse.bacc as bacc
import concourse.tile as tile
from concourse import mybir
from concourse.bass_utils import run_bass_kernel_spmd

F32 = mybir.dt.float32
BF16 = mybir.dt.bfloat16
AF = mybir.ActivationFunctionType
OP = mybir.AluOpType

N = 8192
F_IN = 256
F_OUT = 128
N_CORES = 8
ROWS = N // N_CORES          # 1024 output rows per core
P = 128                      # partitions
JT = N // P                  # 64 j-tiles per core
IT = ROWS // P               # 8 i-tiles per core
SLOPE = 0.2

# Fraction of mask-multiplies routed to GpSimd instead of VectorE (tunable).
GPSIMD_EVERY = 0             # 0 = disabled; k>0 = every k-th tile

LAST_EXEC_TIME_NS = None
LAST_RESULT = None


def _build_program():
    nc = bacc.Bacc("TRN2", target_bir_lowering=False, debug=False,
                   num_devices=N_CORES)

    mask = nc.dram_tensor("mask", [N, ROWS], BF16, kind="ExternalInput")
    xT = nc.dram_tensor("xT", [F_IN, N], BF16, kind="ExternalInput")
    xoT = nc.dram_tensor("xoT", [F_IN, ROWS], BF16, kind="ExternalInput")
    w_in = nc.dram_tensor("W", [F_IN, F_OUT], BF16, kind="ExternalInput")
    asrc = nc.dram_tensor("a_src", [F_OUT, 1], BF16, kind="ExternalInput")
    adst = nc.dram_tensor("a_dst", [F_OUT, 1], BF16, kind="ExternalInput")
    ident = nc.dram_tensor("ident", [P, P], F32, kind="ExternalInput")
    ones_rb = nc.dram_tensor("ones_rb", [1, P], BF16, kind="ExternalInput")
    ones_cb = nc.dram_tensor("ones_cb", [P, 1], BF16, kind="ExternalInput")
    out = nc.dram_tensor("out", [ROWS, F_OUT], F32, kind="ExternalOutput")
    # 2-D shapes: 1-D DRAM outputs fail to load under the axon PJRT path.
    # ExternalOutput doubles as DRAM scratch (Internal DRAM also fails).
    fd_dram = nc.dram_tensor("fd_scratch", [1, N], F32, kind="ExternalOutput")
    den_dram = nc.dram_tensor("den_scratch", [1, ROWS], F32,
                              kind="ExternalOutput")

    with tile.TileContext(nc) as tc:
        with ExitStack() as ctx:
            persist = ctx.enter_context(tc.tile_pool(name="persist", bufs=1))
            opsum = ctx.enter_context(
                tc.tile_pool(name="opsum", bufs=1, space="PSUM"))

            h_sb = persist.tile([P, JT * P], BF16)     # h[j,f] per j-block
            c_bcast = persist.tile([P, ROWS], BF16)    # exp(-0.8 f_src) bcast
            b_col = persist.tile([P, JT], F32)         # exp(f_dst)
            d_col = persist.tile([P, JT], F32)         # exp(0.2 f_dst)
            id_sb = persist.tile([P, P], F32)
            ones_r_sb = persist.tile([1, P], BF16)
            ones_c_sb = persist.tile([P, 1], BF16)
            w_sb = persist.tile([P, 2 * F_OUT], BF16)  # W k-halves
            asrc_sb = persist.tile([F_OUT, 1], BF16)
            adst_sb = persist.tile([F_OUT, 1], BF16)
            inv_col = persist.tile([P, IT], F32)

            nc.sync.dma_start(id_sb[:], ident[:, :])
            nc.sync.dma_start(ones_r_sb[:], ones_rb[:, :])
            nc.sync.dma_start(ones_c_sb[:], ones_cb[:, :])
            nc.sync.dma_start(w_sb[:, 0:F_OUT], w_in[0:P, :])
            nc.sync.dma_start(w_sb[:, F_OUT:2 * F_OUT], w_in[P:2 * P, :])
            nc.sync.dma_start(asrc_sb[:], asrc[:, :])
            nc.sync.dma_start(adst_sb[:], adst[:, :])

            # ---------------- prep: h, hT, f_src, f_dst ----------------
            with ExitStack() as pctx:
                prep = pctx.enter_context(tc.tile_pool(name="prep", bufs=1))
                ppsum = pctx.enter_context(
                    tc.tile_pool(name="ppsum", bufs=2, space="PSUM"))

                xt_sb = prep.tile([P, 2 * N], BF16)    # xT k-halves
                nc.sync.dma_start(xt_sb[:, 0:N], xT[0:P, :])
                nc.sync.dma_start(xt_sb[:, N:2 * N], xT[P:2 * P, :])
                xo_sb = prep.tile([P, 2 * ROWS], BF16)
                nc.sync.dma_start(xo_sb[:, 0:ROWS], xoT[0:P, :])
                nc.sync.dma_start(xo_sb[:, ROWS:2 * ROWS], xoT[P:2 * P, :])

                # h[j,f] blocks (lhsT for the output matmul): 4 blocks/psum
                for q in range(JT // 4):
                    ph = ppsum.tile([P, 512], F32, tag="pp")
                    for u4 in range(4):
                        jt = q * 4 + u4
                        psl = ph[:, u4 * P:(u4 + 1) * P]
                        nc.tensor.matmul(
                            psl, lhsT=xt_sb[:, jt * P:(jt + 1) * P],
                            rhs=w_sb[:, 0:F_OUT], start=True, stop=False)
                        nc.tensor.matmul(
                            psl, lhsT=xt_sb[:, N + jt * P:N + (jt + 1) * P],
                            rhs=w_sb[:, F_OUT:2 * F_OUT], start=False,
                            stop=True)
                    nc.scalar.copy(h_sb[:, q * 512:(q + 1) * 512], ph[:])

                # hT[f,j] (only to compute f_dst)
                hT_sb = prep.tile([P, N], BF16)
                for q in range(N // 512):
                    pt = ppsum.tile([P, 512], F32, tag="pp")
                    nc.tensor.matmul(
                        pt[:], lhsT=w_sb[:, 0:F_OUT],
                        rhs=xt_sb[:, q * 512:(q + 1) * 512],
                        start=True, stop=False)
                    nc.tensor.matmul(
                        pt[:], lhsT=w_sb[:, F_OUT:2 * F_OUT],
                        rhs=xt_sb[:, N + q * 512:N + (q + 1) * 512],
                        start=False, stop=True)
                    nc.scalar.copy(hT_sb[:, q * 512:(q + 1) * 512], pt[:])

                # f_dst row [1, N] -> DRAM -> per-partition layout [128, 64]
                fd_sbuf = prep.tile([1, N], F32)
                for q in range(N // 512):
                    pf = ppsum.tile([1, 512], F32, tag="pp")
                    nc.tensor.matmul(
                        pf[:], lhsT=adst_sb[:],
                        rhs=hT_sb[:, q * 512:(q + 1) * 512],
                        start=True, stop=True)
                    nc.scalar.copy(fd_sbuf[:, q * 512:(q + 1) * 512], pf[:])
                nc.sync.dma_start(fd_dram[:, :], fd_sbuf[:])
                fd_col = prep.tile([P, JT], F32)
                nc.sync.dma_start(
                    fd_col[:], fd_dram.ap()[0].rearrange("(jt p) -> p jt", p=P))
                nc.scalar.activation(b_col[:], fd_col[:], AF.Exp)
                nc.scalar.activation(d_col[:], fd_col[:], AF.Exp, scale=SLOPE)

                # h_ownT[f,i] -> f_src row -> c = exp(-0.8 f_src), broadcast
                hown_sb = prep.tile([P, ROWS], BF16)
                for q in range(ROWS // 512):
                    po = ppsum.tile([P, 512], F32, tag="pp")
                    nc.tensor.matmul(
                        po[:], lhsT=w_sb[:, 0:F_OUT],
                        rhs=xo_sb[:, q * 512:(q + 1) * 512],
                        start=True, stop=False)
                    nc.tensor.matmul(
                        po[:], lhsT=w_sb[:, F_OUT:2 * F_OUT],
                        rhs=xo_sb[:, ROWS + q * 512:ROWS + (q + 1) * 512],
                        start=False, stop=True)
                    nc.scalar.copy(hown_sb[:, q * 512:(q + 1) * 512], po[:])
                c_row = prep.tile([1, ROWS], BF16)
                for q in range(ROWS // 512):
                    pfs = ppsum.tile([1, 512], F32, tag="pp")
                    nc.tensor.matmul(
                        pfs[:], lhsT=asrc_sb[:],
                        rhs=hown_sb[:, q * 512:(q + 1) * 512],
                        start=True, stop=True)
                    nc.scalar.activation(c_row[:, q * 512:(q + 1) * 512],
                                         pfs[:], AF.Exp, scale=-0.8)
                for q in range(ROWS // 512):
                    pcb = ppsum.tile([P, 512], F32, tag="pp")
                    nc.tensor.matmul(
                        pcb[:], lhsT=ones_r_sb[:],
                        rhs=c_row[:, q * 512:(q + 1) * 512],
                        start=True, stop=True)
                    nc.scalar.copy(c_bcast[:, q * 512:(q + 1) * 512], pcb[:])

            # ---------------- main loop over 64 j-tiles ----------------
            with ExitStack() as mctx:
                msk_pool = mctx.enter_context(tc.tile_pool(name="msk", bufs=6))
                m_pool = mctx.enter_context(tc.tile_pool(name="m", bufs=4))
                s_pool = mctx.enter_context(tc.tile_pool(name="s", bufs=4))

                psum_out = opsum.tile([P, ROWS], F32)   # outT accumulator
                psum_den = opsum.tile([1, ROWS], F32)   # denom accumulator

                for jt in range(JT):
                    mk = msk_pool.tile([P, ROWS], BF16, tag="mk")
                    nc.sync.dma_start(mk[:], mask[jt * P:(jt + 1) * P, :])

                    m = m_pool.tile([P, ROWS], BF16, tag="m")
                    nc.vector.tensor_scalar(
                        m[:], c_bcast[:], d_col[:, jt:jt + 1],
                        b_col[:, jt:jt + 1], op0=OP.mult, op1=OP.max)

                    s = s_pool.tile([P, ROWS], BF16, tag="s")
                    if GPSIMD_EVERY and jt % GPSIMD_EVERY == 0:
                        nc.gpsimd.tensor_tensor(s[:], m[:], mk[:], op=OP.mult)
                    else:
                        nc.vector.tensor_tensor(s[:], m[:], mk[:], op=OP.mult)

                    for hh in range(2):
                        sl = slice(hh * 512, (hh + 1) * 512)
                        nc.tensor.matmul(
                            psum_out[:, sl],
                            lhsT=h_sb[:, jt * P:(jt + 1) * P],
                            rhs=s[:, sl],
                            start=(jt == 0), stop=(jt == JT - 1))
                        nc.tensor.matmul(
                            psum_den[:, sl], lhsT=ones_c_sb[:], rhs=s[:, sl],
                            start=(jt == 0), stop=(jt == JT - 1))

            # ---------------- epilogue: normalize + transpose ----------
            with ExitStack() as ectx:
                epi = ectx.enter_context(tc.tile_pool(name="epi", bufs=2))
                epsum = ectx.enter_context(
                    tc.tile_pool(name="epsum", bufs=2, space="PSUM"))

                den_row = epi.tile([1, ROWS], F32, tag="den")
                nc.scalar.copy(den_row[:], psum_den[:])
                nc.sync.dma_start(den_dram[:, :], den_row[:])
                den_col = epi.tile([P, IT], F32, tag="denc")
                nc.sync.dma_start(
                    den_col[:], den_dram.ap()[0].rearrange("(it p) -> p it", p=P))
                nc.vector.reciprocal(inv_col[:], den_col[:])

                outT_sb = epi.tile([P, ROWS], F32, tag="outT")
                nc.scalar.copy(outT_sb[:], psum_out[:])
                for it in range(IT):
                    ptr = epsum.tile([P, P], F32, tag="ep")
                    nc.tensor.transpose(
                        ptr[:], outT_sb[:, it * P:(it + 1) * P], id_sb[:])
                    ot = epi.tile([P, P], F32, tag="ot")
                    nc.vector.tensor_scalar_mul(
                        ot[:], ptr[:], inv_col[:, it:it + 1])
                    nc.sync.dma_start(out[it * P:(it + 1) * P, :], ot[:])

    nc.compile()
    return nc


_PROGRAM = None


def _get_program():
    global _PROGRAM
    if _PROGRAM is None:
        _PROGRAM = _build_program()
    return _PROGRAM


def kernel(x, adj, W, a_src, a_dst):
    global LAST_EXEC_TIME_NS, LAST_RESULT
    x = np.asarray(x, dtype=np.float32)
    adj = np.asarray(adj, dtype=np.float32)
    W = np.asarray(W, dtype=np.float32)
    a_src = np.asarray(a_src, dtype=np.float32).reshape(F_OUT, 1)
    a_dst = np.asarray(a_dst, dtype=np.float32).reshape(F_OUT, 1)

    nc = _get_program()

    bf = ml_dtypes.bfloat16
    xT = np.ascontiguousarray(x.T).astype(bf)
    in_common = {
        "xT": xT,
        "W": W.astype(bf),
        "a_src": a_src.astype(bf),
        "a_dst": a_dst.astype(bf),
        "ident": np.eye(P, dtype=np.float32),
        "ones_rb": np.ones((1, P), dtype=bf),
        "ones_cb": np.ones((P, 1), dtype=bf),
    }
    in_maps = []
    for c in range(N_CORES):
        rows = slice(c * ROWS, (c + 1) * ROWS)
        im = dict(in_common)
        im["mask"] = np.ascontiguousarray(adj[rows, :].T).astype(bf)
        im["xoT"] = np.ascontiguousarray(x[rows, :].T).astype(bf)
        in_maps.append(im)

    res = run_bass_kernel_spmd(nc, in_maps, core_ids=list(range(N_CORES)))
    LAST_EXEC_TIME_NS = res.exec_time_ns
    LAST_RESULT = res
    return np.concatenate(
        [res.results[c]["out"] for c in range(N_CORES)], axis=0)


# revision 10
# speedup vs baseline: 3.5586x; 1.1034x over previous
"""Dense graph-attention layer (GAT) on 8 Trainium2 NeuronCores.

Reference computation (all f32):
    h = x @ W                      # [N, F_OUT]
    f_src = h @ a_src              # [N]
    f_dst = h @ a_dst              # [N]
    e[i,j] = leaky_relu(f_src[i] + f_dst[j], 0.2), masked to -inf where adj==0
    alpha = softmax(e, axis=1)
    out = alpha @ h                # [N, F_OUT]

Sharding: output rows i are sharded across 8 cores (1024 rows each). Each
core receives its slice of adj transposed to [N, 1024] as a bf16 0/1 mask,
so j lands on partitions when tiled — the orientation the alpha @ h
contraction needs.

Key reformulation (exact, not approximate): with softmax's invariance to a
per-row scale, exp(f_src[i]) factors out of both the numerator and the
denominator and cancels. Writing B = exp(f_dst), D = exp(0.2 f_dst),
c = exp(-0.8 f_src):
    exp(leaky_relu(e)) = max(exp(e), exp(0.2 e))          (exp is monotone)
                       = exp(f_src) * max(B[j], c[i] * D[j])
so alpha rows can be computed from s[j,i] = mask[j,i] * max(B[j], c[i]*D[j])
directly. No per-element exp/activation is needed — only a dual-op
tensor_scalar (mult+max against two per-partition vectors) and a mask
multiply, both on VectorE in bf16. PE accumulates outT += h_tile.T @ s and
denom += ones.T @ s across all 64 j-tiles in PSUM. exp() runs only on tiny
f_src/f_dst vectors. f_src/f_dst are computed as x @ (W @ a) with the
weight-only products W @ a_src / W @ a_dst folded on the host, and the
per-j-block h tiles are built inside the main loop so the x @ W matmuls
overlap the mask DMA stream.
"""

import numpy as np
import ml_dtypes
from contextlib import ExitStack

import concourse.bacc as bacc
import concourse.tile as tile
from concourse import mybir
from concourse.bass_utils import run_bass_kernel_spmd

F32 = mybir.dt.float32
BF16 = mybir.dt.bfloat16
AF = mybir.ActivationFunctionType
OP = mybir.AluOpType

N = 8192
F_IN = 256
F_OUT = 128
N_CORES = 8
ROWS = N // N_CORES          # 1024 output rows per core
P = 128                      # partitions
JT = N // P                  # 64 j-tiles per core
IT = ROWS // P               # 8 i-tiles per core
SLOPE = 0.2

LAST_EXEC_TIME_NS = None
LAST_RESULT = None


def _build_program():
    nc = bacc.Bacc("TRN2", target_bir_lowering=False, debug=False,
                   num_devices=N_CORES)

    mask = nc.dram_tensor("mask", [N, ROWS], BF16, kind="ExternalInput")
    xT = nc.dram_tensor("xT", [F_IN, N], BF16, kind="ExternalInput")
    xoT = nc.dram_tensor("xoT", [F_IN, ROWS], BF16, kind="ExternalInput")
    w_in = nc.dram_tensor("W", [F_IN, F_OUT], BF16, kind="ExternalInput")
    # wa_dst = W @ a_dst, wa_src = W @ a_src  (weight-only, host-folded)
    wad = nc.dram_tensor("wa_dst", [F_IN, 1], BF16, kind="ExternalInput")
    was = nc.dram_tensor("wa_src", [F_IN, 1], BF16, kind="ExternalInput")
    ident = nc.dram_tensor("ident", [P, P], F32, kind="ExternalInput")
    ones_rb = nc.dram_tensor("ones_rb", [1, P], BF16, kind="ExternalInput")
    ones_cb = nc.dram_tensor("ones_cb", [P, 1], BF16, kind="ExternalInput")
    out = nc.dram_tensor("out", [ROWS, F_OUT], F32, kind="ExternalOutput")
    # 2-D shapes: 1-D DRAM outputs fail to load under the axon PJRT path.
    # ExternalOutput doubles as DRAM scratch (Internal DRAM also fails).
    fd_dram = nc.dram_tensor("fd_scratch", [1, N], F32, kind="ExternalOutput")
    den_dram = nc.dram_tensor("den_scratch", [1, ROWS], F32,
                              kind="ExternalOutput")

    with tile.TileContext(nc) as tc:
        with ExitStack() as ctx:
            persist = ctx.enter_context(tc.tile_pool(name="persist", bufs=1))
            opsum = ctx.enter_context(
                tc.tile_pool(name="opsum", bufs=1, space="PSUM"))

            xt_sb = persist.tile([P, 2 * N], BF16)     # xT k-halves
            c_bcast = persist.tile([P, ROWS], BF16)    # exp(-0.8 f_src) bcast
            b_col = persist.tile([P, JT], F32)         # exp(f_dst)
            d_col = persist.tile([P, JT], F32)         # exp(0.2 f_dst)
            id_sb = persist.tile([P, P], F32)
            ones_r_sb = persist.tile([1, P], BF16)
            ones_c_sb = persist.tile([P, 1], BF16)
            w_sb = persist.tile([P, 2 * F_OUT], BF16)  # W k-halves
            wad_sb = persist.tile([P, 2], BF16)        # wa_dst k-halves
            was_sb = persist.tile([P, 2], BF16)        # wa_src k-halves
            inv_col = persist.tile([P, IT], F32)

            nc.sync.dma_start(id_sb[:], ident[:, :])
            nc.sync.dma_start(ones_r_sb[:], ones_rb[:, :])
            nc.sync.dma_start(ones_c_sb[:], ones_cb[:, :])
            nc.sync.dma_start(w_sb[:, 0:F_OUT], w_in[0:P, :])
            nc.sync.dma_start(w_sb[:, F_OUT:2 * F_OUT], w_in[P:2 * P, :])
            nc.sync.dma_start(wad_sb[:, 0:1], wad[0:P, :])
            nc.sync.dma_start(wad_sb[:, 1:2], wad[P:2 * P, :])
            nc.sync.dma_start(was_sb[:, 0:1], was[0:P, :])
            nc.sync.dma_start(was_sb[:, 1:2], was[P:2 * P, :])
            nc.sync.dma_start(xt_sb[:, 0:N], xT[0:P, :])
            nc.sync.dma_start(xt_sb[:, N:2 * N], xT[P:2 * P, :])

            # ------------ prep: f_dst, f_src -> B, D, c vectors ---------
            with ExitStack() as pctx:
                prep = pctx.enter_context(tc.tile_pool(name="prep", bufs=1))
                ppsum = pctx.enter_context(
                    tc.tile_pool(name="ppsum", bufs=2, space="PSUM"))

                xo_sb = prep.tile([P, 2 * ROWS], BF16)
                nc.sync.dma_start(xo_sb[:, 0:ROWS], xoT[0:P, :])
                nc.sync.dma_start(xo_sb[:, ROWS:2 * ROWS], xoT[P:2 * P, :])

                # f_dst row = wa_dst.T @ xT  -> [1, N]
                fd_sbuf = prep.tile([1, N], F32)
                for q in range(N // 512):
                    pf = ppsum.tile([1, 512], F32, tag="pp")
                    nc.tensor.matmul(
                        pf[:], lhsT=wad_sb[:, 0:1],
                        rhs=xt_sb[:, q * 512:(q + 1) * 512],
                        start=True, stop=False)
                    nc.tensor.matmul(
                        pf[:], lhsT=wad_sb[:, 1:2],
                        rhs=xt_sb[:, N + q * 512:N + (q + 1) * 512],
                        start=False, stop=True)
                    nc.scalar.copy(fd_sbuf[:, q * 512:(q + 1) * 512], pf[:])
                nc.sync.dma_start(fd_dram[:, :], fd_sbuf[:])
                fd_col = prep.tile([P, JT], F32)
                nc.sync.dma_start(
                    fd_col[:], fd_dram.ap()[0].rearrange("(jt p) -> p jt", p=P))
                nc.scalar.activation(b_col[:], fd_col[:], AF.Exp)
                nc.scalar.activation(d_col[:], fd_col[:], AF.Exp, scale=SLOPE)

                # f_src row = wa_src.T @ xoT -> c = exp(-0.8 f_src), bcast
                c_row = prep.tile([1, ROWS], BF16)
                for q in range(ROWS // 512):
                    pfs = ppsum.tile([1, 512], F32, tag="pp")
                    nc.tensor.matmul(
                        pfs[:], lhsT=was_sb[:, 0:1],
                        rhs=xo_sb[:, q * 512:(q + 1) * 512],
                        start=True, stop=False)
                    nc.tensor.matmul(
                        pfs[:], lhsT=was_sb[:, 1:2],
                        rhs=xo_sb[:, ROWS + q * 512:ROWS + (q + 1) * 512],
                        start=False, stop=True)
                    nc.scalar.activation(c_row[:, q * 512:(q + 1) * 512],
                                         pfs[:], AF.Exp, scale=-0.8)
                for q in range(ROWS // 512):
                    pcb = ppsum.tile([P, 512], F32, tag="pp")
                    nc.tensor.matmul(
                        pcb[:], lhsT=ones_r_sb[:],
                        rhs=c_row[:, q * 512:(q + 1) * 512],
                        start=True, stop=True)
                    nc.scalar.copy(c_bcast[:, q * 512:(q + 1) * 512], pcb[:])

            # ---------------- main loop over 64 j-tiles ----------------
            with ExitStack() as mctx:
                msk_pool = mctx.enter_context(tc.tile_pool(name="msk", bufs=6))
                m_pool = mctx.enter_context(tc.tile_pool(name="m", bufs=4))
                s_pool = mctx.enter_context(tc.tile_pool(name="s", bufs=4))
                h_pool = mctx.enter_context(tc.tile_pool(name="h", bufs=3))
                hpsum = mctx.enter_context(
                    tc.tile_pool(name="hpsum", bufs=2, space="PSUM"))

                psum_out = opsum.tile([P, ROWS], F32)   # outT accumulator
                psum_den = opsum.tile([1, ROWS], F32)   # denom accumulator

                for jt in range(JT):
                    mk = msk_pool.tile([P, ROWS], BF16, tag="mk")
                    nc.sync.dma_start(mk[:], mask[jt * P:(jt + 1) * P, :])

                    # h block for this j-tile: [128 j, 128 f] (bf16 lhsT)
                    hp = hpsum.tile([P, P], F32, tag="hp")
                    nc.tensor.matmul(
                        hp[:], lhsT=xt_sb[:, jt * P:(jt + 1) * P],
                        rhs=w_sb[:, 0:F_OUT], start=True, stop=False)
                    nc.tensor.matmul(
                        hp[:], lhsT=xt_sb[:, N + jt * P:N + (jt + 1) * P],
                        rhs=w_sb[:, F_OUT:2 * F_OUT], start=False, stop=True)
                    hb = h_pool.tile([P, P], BF16, tag="hb")
                    nc.scalar.copy(hb[:], hp[:])

                    m = m_pool.tile([P, ROWS], BF16, tag="m")
                    nc.vector.tensor_scalar(
                        m[:], c_bcast[:], d_col[:, jt:jt + 1],
                        b_col[:, jt:jt + 1], op0=OP.mult, op1=OP.max)

                    s = s_pool.tile([P, ROWS], BF16, tag="s")
                    nc.vector.tensor_tensor(s[:], m[:], mk[:], op=OP.mult)

                    for hh in range(2):
                        sl = slice(hh * 512, (hh + 1) * 512)
                        nc.tensor.matmul(
                            psum_out[:, sl], lhsT=hb[:], rhs=s[:, sl],
                            start=(jt == 0), stop=(jt == JT - 1))
                        nc.tensor.matmul(
                            psum_den[:, sl], lhsT=ones_c_sb[:], rhs=s[:, sl],
                            start=(jt == 0), stop=(jt == JT - 1))

            # ---------------- epilogue: normalize + transpose ----------
            with ExitStack() as ectx:
                epi = ectx.enter_context(tc.tile_pool(name="epi", bufs=2))
                epsum = ectx.enter_context(
                    tc.tile_pool(name="epsum", bufs=2, space="PSUM"))

                den_row = epi.tile([1, ROWS], F32, tag="den")
                nc.scalar.copy(den_row[:], psum_den[:])
                nc.sync.dma_start(den_dram[:, :], den_row[:])
                den_col = epi.tile([P, IT], F32, tag="denc")
                nc.sync.dma_start(
                    den_col[:], den_dram.ap()[0].rearrange("(it p) -> p it", p=P))
                nc.vector.reciprocal(inv_col[:], den_col[:])

                outT_sb = epi.tile([P, ROWS], F32, tag="outT")
                nc.scalar.copy(outT_sb[:], psum_out[:])
                for it in range(IT):
                    ptr = epsum.tile([P, P], F32, tag="ep")
                    nc.tensor.transpose(
                        ptr[:], outT_sb[:, it * P:(it + 1) * P], id_sb[:])
                    ot = epi.tile([P, P], F32, tag="ot")
                    nc.vector.tensor_scalar_mul(
                        ot[:], ptr[:], inv_col[:, it:it + 1])
                    nc.sync.dma_start(out[it * P:(it + 1) * P, :], ot[:])

    nc.compile()
    return nc


_PROGRAM = None


def _get_program():
    global _PROGRAM
    if _PROGRAM is None:
        _PROGRAM = _build_program()
    return _PROGRAM


def kernel(x, adj, W, a_src, a_dst):
    global LAST_EXEC_TIME_NS, LAST_RESULT
    x = np.asarray(x, dtype=np.float32)
    adj = np.asarray(adj, dtype=np.float32)
    W = np.asarray(W, dtype=np.float32)
    a_src = np.asarray(a_src, dtype=np.float32).reshape(F_OUT)
    a_dst = np.asarray(a_dst, dtype=np.float32).reshape(F_OUT)

    nc = _get_program()

    bf = ml_dtypes.bfloat16
    xT = np.ascontiguousarray(x.T).astype(bf)
    in_common = {
        "xT": xT,
        "W": W.astype(bf),
        "wa_dst": (W @ a_dst).reshape(F_IN, 1).astype(bf),
        "wa_src": (W @ a_src).reshape(F_IN, 1).astype(bf),
        "ident": np.eye(P, dtype=np.float32),
        "ones_rb": np.ones((1, P), dtype=bf),
        "ones_cb": np.ones((P, 1), dtype=bf),
    }
    in_maps = []
    for c in range(N_CORES):
        rows = slice(c * ROWS, (c + 1) * ROWS)
        im = dict(in_common)
        im["mask"] = np.ascontiguousarray(adj[rows, :].T).astype(bf)
        im["xoT"] = np.ascontiguousarray(x[rows, :].T).astype(bf)
        in_maps.append(im)

    res = run_bass_kernel_spmd(nc, in_maps, core_ids=list(range(N_CORES)))
    LAST_EXEC_TIME_NS = res.exec_time_ns
    LAST_RESULT = res
    return np.concatenate(
        [res.results[c]["out"] for c in range(N_CORES)], axis=0)
